# revision 6
# baseline (speedup 1.0000x reference)
"""Multi-head self-attention on 8 TRN2 NeuronCores.

Problem: x(4,2048,1024), Wq(8,1024,128), Wk/Wv(1024,128), Wo(1024,1024) fp32.
out = softmax(Q K^T / sqrt(128)) V -> concat heads -> @ Wo.

Sharding: (batch, query-half) across 8 cores — core c handles batch c//2,
query rows [(c%2)*1024, (c%2)*1024+1024). K/V cover the full sequence of the
batch, so each core computes them locally from its x slice; no collectives.

Numerics: scores have std ~1024 and softmax is near-one-hot, so the
x->Q/K->scores chain needs ~fp32 precision. bf16 matmuls with hi/lo split
operands ("split3": Ah*Bh + Ah*Bl + Al*Bh, fp32 PSUM accumulation) give
~5e-6 relative matmul error at 3 cycles/row (native fp32 is 4). The x and
weight splits are precomputed on the host. V/ctx/Wo paths are plain bf16.

Layouts (partition dim first):
  xT (E,S) host-transposed; K^T (O,S) = sum_e Wk[e].T-stationary @ xT[e];
  Q_h^T (O,Sq) likewise (Wq pre-scaled by 1/sqrt(O) on host);
  scores tile (128q, 2048s) = Q^T-slice-stationary @ K^T-moving, fp32 PSUM;
  softmax per q-row: DVE reduce_max(negate) -> ACT exp(bias=-max, accum=den)
  -> DVE x(1/den); P transposed 128x128 via PE to (s,q); ctx^T (O,Sq) =
  V-stationary @ P^T-moving; out^T (E,Sq) = Wo-stationary @ ctx^T-moving;
  host transposes out^T back.
"""
import numpy as np
import ml_dtypes

B, S, E, H, O = 4, 2048, 1024, 8, 128
SQ = S // 2          # query rows per core
NCORES = 8
ET = E // 128        # 8 e-tiles
ST = S // 128        # 16 s-tiles
QT = SQ // 128       # 8 q-tiles

_compiled = None     # (nc,) cache so repeated kernel() calls skip rebuild


def _build():
    import concourse.bass as bass
    import concourse.mybir as mybir
    import concourse.tile as tile
    from concourse import bacc
    from concourse.masks import make_identity

    F32 = mybir.dt.float32
    BF16 = mybir.dt.bfloat16
    PS = bass.MemorySpace.PSUM

    nc = bacc.Bacc("TRN2", target_bir_lowering=False, debug=False,
                   enable_asserts=True)

    d_xkvh = nc.dram_tensor("xkvh", (E, S), BF16, kind="ExternalInput").ap()
    d_xkvl = nc.dram_tensor("xkvl", (E, S), BF16, kind="ExternalInput").ap()
    d_xqh = nc.dram_tensor("xqh", (E, SQ), BF16, kind="ExternalInput").ap()
    d_xql = nc.dram_tensor("xql", (E, SQ), BF16, kind="ExternalInput").ap()
    d_wqh = nc.dram_tensor("wqh", (H, E, O), BF16, kind="ExternalInput").ap()
    d_wql = nc.dram_tensor("wql", (H, E, O), BF16, kind="ExternalInput").ap()
    d_wkh = nc.dram_tensor("wkh", (E, O), BF16, kind="ExternalInput").ap()
    d_wkl = nc.dram_tensor("wkl", (E, O), BF16, kind="ExternalInput").ap()
    d_wvh = nc.dram_tensor("wvh", (E, O), BF16, kind="ExternalInput").ap()
    d_woh = nc.dram_tensor("woh", (H * O, E), BF16, kind="ExternalInput").ap()
    d_out = nc.dram_tensor("outT", (E, SQ), F32, kind="ExternalOutput").ap()

    with tile.TileContext(nc) as tc:
        with (
            tc.tile_pool(name="persist", bufs=1) as persist,
            tc.tile_pool(name="tiny", bufs=24) as tiny,
        ):
            ident = persist.tile([128, 128], BF16, tag="ident")
            make_identity(nc, ident[:])

            wo_sb = persist.tile([128, H, E], BF16, tag="wo")
            nc.sync.dma_start(wo_sb[:], d_woh.rearrange("(h p) e -> p h e", p=128))

            kth = persist.tile([128, S], BF16, tag="kth")
            ktl = persist.tile([128, S], BF16, tag="ktl")
            qth = persist.tile([128, H, SQ], BF16, tag="qth")
            qtl = persist.tile([128, H, SQ], BF16, tag="qtl")
            v_sb = persist.tile([128, ST, O], BF16, tag="v")

            # ---------------- prologue: K^T, V, Q^T projections ----------
            with (
                tc.tile_pool(name="xp", bufs=1) as xp,
                tc.tile_pool(name="pp", bufs=1, space=PS) as pp,
                tc.tile_pool(name="vp", bufs=2, space=PS) as vp,
                tc.tile_pool(name="qp", bufs=1, space=PS) as qp,
            ):
                xkvh = xp.tile([128, ET, S], BF16, tag="xkvh")
                xkvl = xp.tile([128, ET, S], BF16, tag="xkvl")
                for e in range(ET):
                    nc.sync.dma_start(xkvh[:, e, :], d_xkvh[e * 128:(e + 1) * 128, :])
                    nc.sync.dma_start(xkvl[:, e, :], d_xkvl[e * 128:(e + 1) * 128, :])
                wkh = xp.tile([128, ET, O], BF16, tag="wkh")
                wkl = xp.tile([128, ET, O], BF16, tag="wkl")
                wvh = xp.tile([128, ET, O], BF16, tag="wvh")
                nc.sync.dma_start(wkh[:], d_wkh.rearrange("(t p) o -> p t o", p=128))
                nc.sync.dma_start(wkl[:], d_wkl.rearrange("(t p) o -> p t o", p=128))
                nc.sync.dma_start(wvh[:], d_wvh.rearrange("(t p) o -> p t o", p=128))

                # K^T (128o x 2048s), split3 accumulation over e
                kt_ps = pp.tile([128, S], F32, tag="kt")
                for e in range(ET):
                    for ti, (w, xx) in enumerate(
                        ((wkh, xkvh), (wkh, xkvl), (wkl, xkvh))
                    ):
                        for c in range(S // 512):
                            nc.tensor.matmul(
                                kt_ps[:, c * 512:(c + 1) * 512],
                                w[:, e, :],
                                xx[:, e, c * 512:(c + 1) * 512],
                                start=(e == 0 and ti == 0),
                                stop=(e == ET - 1 and ti == 2),
                            )
                nc.scalar.copy(kth[:], kt_ps[:])
                nc.vector.tensor_sub(ktl[:], kt_ps[:], kth[:])

                # V (s-part tiles), plain bf16
                for st in range(ST):
                    v_ps = vp.tile([128, O], F32, tag="vps")
                    for e in range(ET):
                        nc.tensor.matmul(
                            v_ps[:],
                            xkvh[:, e, st * 128:(st + 1) * 128],
                            wvh[:, e, :],
                            start=(e == 0),
                            stop=(e == ET - 1),
                        )
                    nc.vector.tensor_copy(v_sb[:, st, :], v_ps[:])

                # Q^T per head (Wq pre-scaled by 1/sqrt(O) on host)
                xqh = xp.tile([128, ET, SQ], BF16, tag="xqh")
                xql = xp.tile([128, ET, SQ], BF16, tag="xql")
                for e in range(ET):
                    nc.sync.dma_start(xqh[:, e, :], d_xqh[e * 128:(e + 1) * 128, :])
                    nc.sync.dma_start(xql[:, e, :], d_xql[e * 128:(e + 1) * 128, :])
                wqh = xp.tile([128, H, ET, O], BF16, tag="wqh")
                wql = xp.tile([128, H, ET, O], BF16, tag="wql")
                nc.sync.dma_start(
                    wqh[:], d_wqh.rearrange("h (t p) o -> p h t o", p=128))
                nc.sync.dma_start(
                    wql[:], d_wql.rearrange("h (t p) o -> p h t o", p=128))

                for h in range(H):
                    q_ps = qp.tile([128, SQ], F32, tag="qtps")
                    for e in range(ET):
                        for ti, (w, xx) in enumerate(
                            ((wqh, xqh), (wqh, xql), (wql, xqh))
                        ):
                            for c in range(SQ // 512):
                                nc.tensor.matmul(
                                    q_ps[:, c * 512:(c + 1) * 512],
                                    w[:, h, e, :],
                                    xx[:, e, c * 512:(c + 1) * 512],
                                    start=(e == 0 and ti == 0),
                                    stop=(e == ET - 1 and ti == 2),
                                )
                    nc.scalar.copy(qth[:, h, :], q_ps[:])
                    nc.vector.tensor_sub(qtl[:, h, :], q_ps[:], qth[:, h, :])

            # ---------------- main: per-head attention ------------------
            with (
                tc.tile_pool(name="p_pool", bufs=12) as p_pool,
                tc.tile_pool(name="pt_pool", bufs=1) as pt_pool,
                tc.tile_pool(name="ctx_pool", bufs=H) as ctx_pool,
                tc.tile_pool(name="s_ps", bufs=1, space=PS) as s_psp,
                tc.tile_pool(name="pt_ps", bufs=2, space=PS) as pt_psp,
                tc.tile_pool(name="acc_ps", bufs=1, space=PS) as acc_psp,
                tc.tile_pool(name="o_sb", bufs=2) as o_sbp,
            ):
                ctxs = []
                for h in range(H):
                    ps = []
                    for qt in range(QT):
                        s_ps = s_psp.tile([128, S], F32, tag="s")
                        # split3 scores, stationary-major order
                        for ti, (qq, kk) in enumerate(
                            ((qth, kth), (qth, ktl), (qtl, kth))
                        ):
                            for c in range(S // 512):
                                nc.tensor.matmul(
                                    s_ps[:, c * 512:(c + 1) * 512],
                                    qq[:, h, qt * 128:(qt + 1) * 128],
                                    kk[:, c * 512:(c + 1) * 512],
                                    start=(ti == 0),
                                    stop=(ti == 2),
                                )
                        negmax = tiny.tile([128, 1], F32, tag="negmax")
                        nc.vector.reduce_max(
                            out=negmax[:], in_=s_ps[:],
                            axis=mybir.AxisListType.X, negate=True,
                        )
                        p_qt = p_pool.tile([128, S], BF16, tag="p")
                        den = tiny.tile([128, 1], F32, tag="den")
                        nc.scalar.activation(
                            p_qt[:], s_ps[:], mybir.ActivationFunctionType.Exp,
                            bias=negmax[:], scale=1.0, accum_out=den[:],
                        )
                        invden = tiny.tile([128, 1], F32, tag="invden")
                        nc.vector.reciprocal(invden[:], den[:])
                        nc.vector.tensor_scalar_mul(p_qt[:], p_qt[:], invden[:])
                        ps.append(p_qt)

                    # transpose P: per s-tile, 8 q-tile blocks into one bank
                    pt_h = pt_pool.tile([128, ST, SQ], BF16, tag="pt")
                    for st in range(ST):
                        pt_ps = pt_psp.tile([128, QT, 128], BF16, tag="ptps")
                        for qt in range(QT):
                            nc.tensor.transpose(
                                pt_ps[:, qt, :],
                                ps[qt][:, st * 128:(st + 1) * 128],
                                ident[:],
                            )
                        cp = nc.scalar.copy if st % 2 else nc.vector.tensor_copy
                        cp(
                            pt_h[:, st, :],
                            pt_ps[:].rearrange("p a b -> p (a b)"),
                        )

                    # ctx^T (o-part, q-free) accumulated over s-tiles
                    ct_ps = acc_psp.tile([128, SQ], F32, tag="acc")
                    for qc in range(SQ // 512):
                        for st in range(ST):
                            nc.tensor.matmul(
                                ct_ps[:, qc * 512:(qc + 1) * 512],
                                v_sb[:, st, :],
                                pt_h[:, st, qc * 512:(qc + 1) * 512],
                                start=(st == 0),
                                stop=(st == ST - 1),
                            )
                    ctx_h = ctx_pool.tile([128, SQ], BF16, tag="ctx")
                    nc.vector.tensor_copy(ctx_h[:], ct_ps[:])
                    ctxs.append(ctx_h)

                # ---------------- out^T = Wo^T-blocks @ ctx^T ------------
                for e in range(ET):
                    o_ps = acc_psp.tile([128, SQ], F32, tag="acc")
                    for qc in range(SQ // 512):
                        for h in range(H):
                            nc.tensor.matmul(
                                o_ps[:, qc * 512:(qc + 1) * 512],
                                wo_sb[:, h, e * 128:(e + 1) * 128],
                                ctxs[h][:, qc * 512:(qc + 1) * 512],
                                start=(h == 0),
                                stop=(h == H - 1),
                            )
                    o_sb = o_sbp.tile([128, SQ], F32, tag="osb")
                    nc.vector.tensor_copy(o_sb[:], o_ps[:])
                    nc.sync.dma_start(d_out[e * 128:(e + 1) * 128, :], o_sb[:])

    nc.compile()
    return nc


def _split(a):
    """fp32 -> (hi, lo) bf16 pair with hi + lo ~= a."""
    hi = a.astype(ml_dtypes.bfloat16)
    lo = (a - hi.astype(np.float32)).astype(ml_dtypes.bfloat16)
    return hi, lo


def kernel(x, Wq, Wk, Wv, Wo):
    global _compiled
    from concourse.bass_utils import run_bass_kernel_spmd

    if _compiled is None:
        _compiled = _build()
    nc = _compiled

    scale = np.float32(1.0 / np.sqrt(O))
    wqh, wql = _split(Wq.astype(np.float32) * scale)
    wkh, wkl = _split(Wk.astype(np.float32))
    wvh = Wv.astype(ml_dtypes.bfloat16)
    woh = Wo.astype(ml_dtypes.bfloat16)

    in_maps = []
    for c in range(NCORES):
        b, half = divmod(c, 2)
        xT = np.ascontiguousarray(x[b].T)          # (E, S) fp32
        xh, xl = _split(xT)
        in_maps.append({
            "xkvh": xh, "xkvl": xl,
            "xqh": np.ascontiguousarray(xh[:, half * SQ:(half + 1) * SQ]),
            "xql": np.ascontiguousarray(xl[:, half * SQ:(half + 1) * SQ]),
            "wqh": wqh, "wql": wql,
            "wkh": wkh, "wkl": wkl, "wvh": wvh, "woh": woh,
        })

    res = run_bass_kernel_spmd(nc, in_maps, core_ids=list(range(NCORES)))

    out = np.empty((B, S, E), dtype=np.float32)
    for c in range(NCORES):
        b, half = divmod(c, 2)
        out[b, half * SQ:(half + 1) * SQ, :] = res.results[c]["outT"].T
    return out
